# revision 35
# baseline (speedup 1.0000x reference)
"""Trainium2 Bass kernel for nn_MultiHeadAttention (B=4, N=2048, E=768, H=8).

Sharding: 8 cores = 4 batches x 2 head-halves (4 heads each). Each core
computes QKV projections for its head slice, attention, and a partial output
projection; the host sums the two partials per batch and adds bo (+ bv@Wo.T,
which passes through softmax's convex weights unchanged).

fp8 DoubleRow scheme (cost model: DR matmul = 0.5 cyc/out-row vs 1.0 bf16,
and each DR matmul contracts 2 chunks at once => 4x over bf16 per term):
  - QKV projections: x and weights quantized to fp8e4m3 (weights scaled x16)
    plus unscaled fp8 residuals. Q/K biases via DVE; V bias on the host.
  - scores: DR with lhsT chunks (k8, kr) against a stride-0-repeated q8 rhs:
    computes (k8+kr).q8 = k.q8 at 0.5 cyc/row -- same one-factor-quantized
    error structure as a bf16 scores matmul, half the PE time.
  - softmax: ACT exp with scale=SC/256 (weights x16 twice), output fp8e4
    directly (Et8). Softmax denominator via ones-column of the V tile.
  - PV: DR pairs over key-chunks: Et8@(v8 + vr), 2 terms, f32 PSUM accum.
  - normalization: reciprocal of the ones-column row, partition-broadcast
    via a 1-row f32r PE matmul (213ns), DVE multiply straight from two PSUM
    operands -> bf16 c; Pool converts to u8/ur fp8 pair for the O projection.
  - O projection: DR pairs over heads, 3 terms u8@wo8 + ur@wo8 + u8@wor;
    result scaled by 1/256 at the bf16 output copy.

Schedule: ACT (exp) is the bottleneck engine (~133us busy). The emission is
a software pipeline over 16 blocks (head h = b%4, query quarter iq = b//4,
iq-major) with 8 key-pair steps each: every step emits S (2 DR mm) + exp,
then drains a paced PV queue (age >= 2 steps, rate ramps 0/1/2 per step) and
due-listed fillers (K/Q projections, V chunks, O-projection singles), sized
so PE (~105us busy) stays just ahead of ACT without starving it.
"""

import os
import sys

for _p in (
    "/root/.axon_site",
    "/root/.axon_site/_ro/trn_rl_repo",
    "/root/.axon_site/_ro/pypackages",
    "/opt/trn_rl_repo",
):
    if os.path.isdir(_p) and _p not in sys.path:
        sys.path.append(_p)

from collections import deque
from contextlib import ExitStack

import ml_dtypes
import numpy as np

import concourse.bass as bass
import concourse.tile as tile
from concourse import mybir
from concourse.bass_utils import run_bass_kernel_spmd

BF16 = ml_dtypes.bfloat16
FP8 = ml_dtypes.float8_e4m3
E = 768
NT = 2048  # tokens
H = 8
D = 96
HC = 4  # heads per core
KC = 6  # 128-chunks over E
SC = 1.0 / float(np.sqrt(D))
SW = 16.0  # fp8 weight scale
QK_TERMS = 2  # 3 = clean (x8w8 + xrw8 + x8wr); 2 = x8w8 + x8wr

_NC_CACHE = {}

DR = mybir.MatmulPerfMode.DoubleRow


def _build_bass():
    f32 = mybir.dt.float32
    f32r = mybir.dt.float32r
    bf = mybir.dt.bfloat16
    f8 = mybir.dt.float8e4
    nc = bass.Bass(trn_type="TRN2", target_bir_lowering=False, debug=False)

    x8_d = nc.dram_tensor("x8", [128, KC, NT], f8, kind="ExternalInput").ap()
    xr_d = nc.dram_tensor("xr", [128, KC, NT], f8, kind="ExternalInput").ap()
    wqp_d = nc.dram_tensor("wqp", [128, KC, 768], f8, kind="ExternalInput").ap()
    wkp_d = nc.dram_tensor("wkp", [128, KC, 768], f8, kind="ExternalInput").ap()
    # wvp: [wv8 | wvr] packed as 768B rows for full-rate DMA
    wvp_d = nc.dram_tensor("wvp", [128, KC, 768], f8, kind="ExternalInput").ap()
    wop_d = nc.dram_tensor("wop", [128, HC, 2 * E], f8, kind="ExternalInput").ap()
    bqk_d = nc.dram_tensor("bqk", [1, 2, HC, D], f8, kind="ExternalInput").ap()
    yT_d = nc.dram_tensor("yT", [128, KC, NT], bf, kind="ExternalOutput").ap()
    y3a_d = nc.dram_tensor("y3a", [128, KC, 512], bf, kind="ExternalOutput").ap()

    Exp = mybir.ActivationFunctionType.Exp
    ESCALE = float(SC / (2 * SW * SW))  # /2: stride-0-doubled k8 lhsT
    YSCALE = 1.0 / (SW * SW)

    with tile.TileContext(nc) as tc, ExitStack() as ctx:
        consts = ctx.enter_context(tc.tile_pool(name="consts", bufs=1))
        big = ctx.enter_context(tc.tile_pool(name="big", bufs=1))

        wqp_sb = consts.tile([128, KC, 768], f8)
        wkp_sb = consts.tile([128, KC, 768], f8)
        wvp_sb = consts.tile([128, 2 * KC, 384], f8)
        wop_sb = consts.tile([128, HC, 2 * E], f8)
        bqk_sb = consts.tile([1, 2, HC, D], f8)
        wq8_sb = wqp_sb[:, :, 0:384]
        wqr_sb = wqp_sb[:, :, 384:768]
        wk8_sb = wkp_sb[:, :, 0:384]
        wkr_sb = wkp_sb[:, :, 384:768]
        wv8_sb = wvp_sb[:, 0:KC, :]
        wvr_sb = wvp_sb[:, KC : 2 * KC, :]
        wo8_sb = wop_sb[:, :, 0:E]
        wor_sb = wop_sb[:, :, E : 2 * E]
        zcol = consts.tile([128, 1], f32)  # exp bias (zeros)
        onesf8 = consts.tile([1, 512], f8)  # rhs for the bias aug matmul
        ones_row = consts.tile([1, 128], f32)  # lhsT for PE warmup
        ones_row_r = consts.tile([1, 128], f32r)  # f32r lhsT for broadcast

        nc.vector.memset(zcol, 0.0)
        nc.vector.memset(ones_row, 1.0)
        nc.vector.memset(onesf8, 1.0)
        ones2 = bass.AP(
            tensor=onesf8.tensor,
            offset=onesf8.offset,
            ap=[list(onesf8.ap[0]), [0, 2], list(onesf8.ap[-1])],
        )
        with nc.allow_low_precision(reason="f32r ones for broadcast matmul"):
            nc.vector.tensor_copy(ones_row_r, ones_row)

        # ACT touch of zcol (used as the exp bias operand)
        scratch_a = consts.tile([1, 1], f32)
        nc.scalar.copy(scratch_a, zcol[0:1, :])

        q8_sb = big.tile([128, HC, NT], f8)  # [d(pad 128), head, token]
        # K fp8, single plane: the scores DR matmul doubles it via a
        # stride-0 chunk (S = 2*k8.q8, folded into ESCALE). K projections
        # contract x8+xr (terms3), so k8's only error is its own fp8 round.
        kp_sb = big.tile([128, HC, NT], f8)
        # v, ones-augmented and zero-padded: [token%128, jc, head, 128]
        # cols 0:96 = v8, col 96 = 1.0 (softmax row-sum), 97:128 = 0
        v8_sb = big.tile([128, 16, HC, 128], f8)
        vr_sb = big.tile([128, 16, HC, 128], f8)
        out8_sb = big.tile([128, HC, NT], f8)  # [headdim(pad 128), head, token]
        outr_sb = big.tile([128, HC, NT], f8)
        y3a_sb = big.tile([128, KC, 512], bf)  # quarter-3 head-pair-01 partial
        x8_sb = big.tile([128, KC, NT], f8)
        xr_sb = big.tile([128, KC, NT], f8)
        ypool = ctx.enter_context(tc.tile_pool(name="ypool", bufs=2))

        nc.gpsimd.memset(v8_sb[:, :, :, 96:97], 1.0)
        nc.gpsimd.memset(v8_sb[:, :, :, 97:128], 0.0)
        nc.gpsimd.memset(vr_sb[:, :, :, 96:128], 0.0)

        with (
            tc.tile_pool(name="pss", bufs=2, space="PSUM") as pss,
            tc.tile_pool(name="psu", bufs=2, space="PSUM") as psu,
            tc.tile_pool(name="epool", bufs=42) as epool,
            tc.tile_pool(name="npool", bufs=2) as npool,
            tc.tile_pool(name="ps1", bufs=2, space="PSUM") as ps1,
        ):
            # ---------------- DMA issue (transfers serialize; order by
            # first-use on the critical path to the first exp) -------------
            nc.sync.dma_start(x8_sb[:, :, 0:512], x8_d[:, :, 0:512])
            nc.sync.dma_start(wkp_sb, wkp_d)
            nc.sync.dma_start(bqk_sb, bqk_d)
            nc.sync.dma_start(xr_sb[:, 0:3, 0:512], xr_d[:, 0:3, 0:512])
            nc.sync.dma_start(xr_sb[:, 3:KC, 0:512], xr_d[:, 3:KC, 0:512])
            nc.sync.dma_start(wqp_sb[:, 0:2, :], wqp_d[:, 0:2, :])
            nc.sync.dma_start(wqp_sb[:, 2:4, :], wqp_d[:, 2:4, :])
            nc.sync.dma_start(wqp_sb[:, 4:KC, :], wqp_d[:, 4:KC, :])
            nc.sync.dma_start(x8_sb[:, :, 512:1024], x8_d[:, :, 512:1024])
            nc.sync.dma_start(xr_sb[:, :, 512:1024], xr_d[:, :, 512:1024])
            nc.sync.dma_start(x8_sb[:, :, 1024:1536], x8_d[:, :, 1024:1536])
            nc.sync.dma_start(xr_sb[:, :, 1024:1536], xr_d[:, :, 1024:1536])
            nc.sync.dma_start(x8_sb[:, :, 1536:NT], x8_d[:, :, 1536:NT])
            nc.sync.dma_start(xr_sb[:, :, 1536:NT], xr_d[:, :, 1536:NT])
            # wvp as 768B rows: SBUF view [128, KC, 768] over the
            # [128, 2*KC, 384] tile (contiguous per partition).
            wvp_flat = bass.AP(
                tensor=wvp_sb.tensor,
                offset=wvp_sb.offset,
                ap=[list(wvp_sb.ap[0]), [768, KC], [1, 768]],
            )
            nc.sync.dma_start(wvp_flat, wvp_d)
            nc.sync.dma_start(wop_sb, wop_d)

            # ---------------- PE warmup: back-to-back matmuls keep the PE
            # p-state ramp alive until the first weights arrive (~4.7us), so
            # the first projections run at full speed ----------
            for i in range(28):
                wt = ps1.tile([128, 64], f32, tag="p1", name=f"warm{i}")
                nc.tensor.matmul(
                    wt, ones_row, ones_row[0:1, 0:64], start=True, stop=True
                )

            # ---------------- building blocks ----------------
            if QK_TERMS == 3:
                qk_pairs = [
                    (x8_sb, wq8_sb, wk8_sb),
                    (xr_sb, wq8_sb, wk8_sb),
                    (x8_sb, wqr_sb, wkr_sb),
                ]
            else:
                qk_pairs = [
                    (x8_sb, wq8_sb, wk8_sb),
                    (x8_sb, wqr_sb, wkr_sb),
                ]
            n_qk = 3 * len(qk_pairs)

            def one_proj(h, i, wsel, act_bias=False):
                dsl = slice(96 * h, 96 * h + 96)
                isl = slice(512 * i, 512 * i + 512)
                pq = ps1.tile([128, 512], f32, tag="p1", name=f"pq{wsel}_{h}_{i}")
                # Q bias via a 1-partition DR aug term (b8,br rows x ones).
                # K needs NO bias: softmax is shift-invariant per query, so
                # (q+bq).(k+bk) and (q+bq).k give identical attention weights.
                if wsel == 1:
                    nc.tensor.matmul(
                        pq[0:96, :],
                        bqk_sb[0:1, :, h, :],
                        ones2,
                        start=True,
                        stop=False,
                        perf_mode=DR,
                    )
                    pairs = [(x8_sb, None, wq8_sb), (x8_sb, None, wqr_sb)]
                else:
                    # K: full terms3 (x8.wk8 + xr.wk8 + x8.wkr) so k8's only
                    # error is its own fp8 rounding (no residual plane).
                    pairs = [(x8_sb, None, wk8_sb), (xr_sb, None, wk8_sb),
                             (x8_sb, None, wkr_sb)]
                nmm = 3 * len(pairs)
                i_mm = 0
                for c in range(3):
                    csl = slice(2 * c, 2 * c + 2)
                    for pair in pairs:
                        xsb, w = pair[0], pair[2]
                        nc.tensor.matmul(
                            pq[0:96, :],
                            w[:, csl, dsl],
                            xsb[:, csl, isl],
                            start=(i_mm == 0 and wsel == 2),
                            stop=(i_mm == nmm - 1),
                            perf_mode=DR,
                        )
                        i_mm += 1
                Copyf = mybir.ActivationFunctionType.Copy
                dst = kp_sb[0:96, h, isl] if wsel == 2 else q8_sb[0:96, h, isl]
                if act_bias:  # prologue: ACT is idle, shortens the chain
                    nc.scalar.activation(dst, pq[0:96, :], Copyf)
                else:
                    nc.vector.tensor_copy(dst, pq[0:96, :])

            def v_chunk(jc):
                # term order tracks DMA arrival: x8+wv8, x8+wvr, xr+wv8
                ksl = slice(128 * jc, 128 * jc + 128)
                pv = ps1.tile([128, HC, D], f32, tag="p1", name=f"pv{jc}")
                mms = [(x8_sb[:, 2 * c : 2 * c + 2, ksl],
                        wv8_sb[:, 2 * c : 2 * c + 2, :]) for c in range(3)]
                mms += [(x8_sb[:, 2 * c : 2 * c + 2, ksl],
                         wvr_sb[:, 2 * c : 2 * c + 2, :]) for c in range(3)]
                mms += [(xr_sb[:, 2 * c : 2 * c + 2, ksl],
                         wv8_sb[:, 2 * c : 2 * c + 2, :]) for c in range(3)]
                for i_mm, (lhs, rhs) in enumerate(mms):
                    nc.tensor.matmul(
                        pv, lhs, rhs,
                        start=(i_mm == 0),
                        stop=(i_mm == len(mms) - 1),
                        perf_mode=DR,
                    )
                nc.vector.tensor_copy(v8_sb[:, jc, :, 0:96], pv)
                nc.vector.scalar_tensor_tensor(
                    vr_sb[:, jc, :, 0:96],
                    pv,
                    1.0,
                    v8_sb[:, jc, :, 0:96],
                    mybir.AluOpType.mult,
                    mybir.AluOpType.subtract,
                )

            # ---------------- pipeline state ----------------
            blocks = [(b % 4, b // 4) for b in range(16)]  # (h, iq) iq-major
            qreps = {}

            def qrep_of(b):
                if b not in qreps:
                    h, iq = blocks[b]
                    qs = q8_sb[0:96, h, 512 * iq : 512 * iq + 512]
                    qreps[b] = bass.AP(
                        tensor=qs.tensor,
                        offset=qs.offset,
                        ap=[list(qs.ap[0]), [0, 2], list(qs.ap[-1])],
                    )
                return qreps[b]

            et_tiles = {}  # (b, jp) -> Et AP
            ut_tiles = {}  # b -> UT AP
            step_no = [0]
            pv_ready = deque()  # (b, jp, emit_step)
            v_done = [False] * 16

            def emit_S_exp(b, jp):
                h, iq = blocks[b]
                S = pss.tile([128, 2, 512], f32, tag="s", name=f"S{b}_{jp}")
                for n in range(2):
                    jc = 2 * jp + n
                    ks = kp_sb[0:96, h, 128 * jc : 128 * jc + 128]
                    krep = bass.AP(
                        tensor=ks.tensor,
                        offset=ks.offset,
                        ap=[list(ks.ap[0]), [0, 2], list(ks.ap[-1])],
                    )
                    nc.tensor.matmul(
                        S[:, n, :],
                        krep,
                        qrep_of(b),
                        start=True,
                        stop=True,
                        perf_mode=DR,
                    )
                Et = epool.tile([128, 2, 512], f8, tag="e", name=f"Et{b}_{jp}")
                nc.scalar.activation(Et, S, Exp, bias=zcol, scale=ESCALE)
                et_tiles[(b, jp)] = Et
                pv_ready.append((b, jp, step_no[0]))

            def emit_PV(b, jp):
                h, iq = blocks[b]
                # JIT safety: ensure v chunks for this jp exist
                for jc in (2 * jp, 2 * jp + 1):
                    if not v_done[jc]:
                        v_chunk(jc)
                        v_done[jc] = True
                if b not in ut_tiles:
                    ut_tiles[b] = psu.tile([128, 512], f32, tag="u", name=f"UT{b}")
                UT = ut_tiles[b]
                Et = et_tiles.pop((b, jp))
                nc.tensor.matmul(
                    UT,
                    v8_sb[:, 2 * jp : 2 * jp + 2, h, :],
                    Et,
                    start=(jp == 0),
                    stop=False,
                    perf_mode=DR,
                )
                nc.tensor.matmul(
                    UT,
                    vr_sb[:, 2 * jp : 2 * jp + 2, h, :],
                    Et,
                    start=False,
                    stop=(jp == 7),
                    perf_mode=DR,
                )

            o_fill = deque()  # budget-driven O-projection items

            def push_oproj_quarter(q):
                # O projection for query quarter q as fine-grained items.
                isl = slice(512 * q, 512 * q + 512)
                state = {}

                def ysb():
                    if "y" not in state:
                        state["y"] = ypool.tile(
                            [128, KC, 512], bf, tag="ysb", name=f"ysbq{q}"
                        )
                    return state["y"]

                for mc in range(KC):
                    st = {}

                    def ensure_py(mc=mc, st=st):
                        if "py" not in st:
                            st["py"] = ps1.tile(
                                [128, 512], f32, tag="p1", name=f"pyq{q}mc{mc}"
                            )
                        return st["py"]

                    mm_list = []
                    for p in range(2):
                        hsl = slice(2 * p, 2 * p + 2)
                        for wsb, osb in (
                            (wo8_sb, out8_sb),
                            (wo8_sb, outr_sb),
                            (wor_sb, out8_sb),
                        ):
                            mm_list.append((hsl, wsb, osb))

                    def mk(i, hsl, wsb, osb, mc=mc, ensure_py=ensure_py):
                        def f():
                            nc.tensor.matmul(
                                ensure_py(),
                                wsb[:, hsl, 128 * mc : 128 * mc + 128],
                                osb[:, hsl, isl],
                                start=(i == 0),
                                stop=(i == 5),
                                perf_mode=DR,
                            )
                        return f

                    for i, a in enumerate(mm_list):
                        o_fill.append((107, mk(i, *a)))

                    def fin(mc=mc, ensure_py=ensure_py):
                        nc.vector.tensor_scalar_mul(
                            ysb()[:, mc, :], ensure_py(), YSCALE
                        )
                    o_fill.append((0, fin))
                    if mc % 2 == 1:
                        def dmap(mc=mc):
                            nc.sync.dma_start(
                                yT_d[:, mc - 1 : mc + 1, isl],
                                ysb()[:, mc - 1 : mc + 1, :],
                            )
                        o_fill.append((0, dmap))

            def push_oproj_q3_pair01():
                # quarter 3, head pair (0,1) partial -> y3a_sb
                isl = slice(1536, 2048)
                for mc in range(KC):
                    st = {}

                    def ensure_py(mc=mc, st=st):
                        if "py" not in st:
                            st["py"] = ps1.tile(
                                [128, 512], f32, tag="p1", name=f"pyq3a{mc}"
                            )
                        return st["py"]

                    terms = [(wo8_sb, out8_sb), (wo8_sb, outr_sb),
                             (wor_sb, out8_sb)]

                    def mk(i, wsb, osb, mc=mc, ensure_py=ensure_py):
                        def f():
                            nc.tensor.matmul(
                                ensure_py(),
                                wsb[:, 0:2, 128 * mc : 128 * mc + 128],
                                osb[:, 0:2, isl],
                                start=(i == 0),
                                stop=(i == 2),
                                perf_mode=DR,
                            )
                        return f

                    for i, a in enumerate(terms):
                        o_fill.append((107, mk(i, *a)))

                    def fin(mc=mc, ensure_py=ensure_py):
                        nc.vector.tensor_scalar_mul(
                            y3a_sb[:, mc, :], ensure_py(), YSCALE
                        )
                    o_fill.append((0, fin))
                o_fill.append((0, lambda: nc.sync.dma_start(y3a_d, y3a_sb)))

            def emit_norm(b):
                h, iq = blocks[b]
                UT = ut_tiles.pop(b)
                rr = npool.tile([1, 512], f32r, tag="rr", name=f"rr{b}")
                rbp = ps1.tile([128, 512], f32, tag="p1", name=f"rbp{b}")
                rb = npool.tile([128, 512], f32, tag="rb", name=f"rb{b}")
                cn = npool.tile([128, 512], bf, tag="cn", name=f"cn{b}")
                halves = (slice(0, 512),)
                for hs in halves:
                    isl = slice(512 * iq + hs.start, 512 * iq + hs.stop)
                    with nc.allow_low_precision(reason="f32r broadcast of 1/r"):
                        nc.vector.reciprocal(rr[:, hs], UT[96:97, hs])
                    # partition-broadcast via 1-row f32r matmul (1 cyc/row)
                    nc.tensor.matmul(
                        rbp[:, hs],
                        ones_row_r,
                        rr[0:1, hs],
                        start=True,
                        stop=True,
                    )
                    nc.vector.tensor_copy(rb[:, hs], rbp[:, hs])
                    nc.vector.tensor_mul(cn[:, hs], UT[:, hs], rb[:, hs])
                    if b == 15:  # DVE: Pool's chain would sit on the tail
                        nc.vector.tensor_copy(out8_sb[:, h, isl], cn[:, hs])
                        nc.vector.scalar_tensor_tensor(
                            outr_sb[:, h, isl], cn[:, hs], 1.0,
                            out8_sb[:, h, isl],
                            mybir.AluOpType.mult, mybir.AluOpType.subtract,
                        )
                    else:
                        nc.gpsimd.tensor_copy(out8_sb[:, h, isl], cn[:, hs])
                        nc.gpsimd.tensor_tensor(
                            outr_sb[:, h, isl], cn[:, hs], out8_sb[:, h, isl],
                            mybir.AluOpType.subtract,
                        )
                # completed quarter? push O-projection work
                if h == 3 and iq < 3:
                    push_oproj_quarter(iq)
                if h == 1 and iq == 3:
                    push_oproj_q3_pair01()

            norm_due = {}  # block whose norm should be emitted -> True

            def drain_pv(rate):
                popped = 0
                while popped < rate and pv_ready:
                    b, jp, es = pv_ready[0]
                    if step_no[0] - es < 2:
                        break
                    pv_ready.popleft()
                    emit_PV(b, jp)
                    if jp == 7:
                        norm_due[b] = True
                    popped += 1

            def drain_norms():
                for b in sorted(norm_due):
                    del norm_due[b]
                    emit_norm(b)

            def drain_ofill(budget):
                while o_fill and budget >= 0:
                    c, f = o_fill.popleft()
                    f()
                    budget -= c

            # ---------------- due-listed fillers per block ----------------
            def K_proj(h):
                return [lambda i=i, h=h: one_proj(h, i, 2) for i in range(4)]

            def Q_proj(h, iq):
                return [lambda: one_proj(h, iq, 1)]

            def V(jc):
                def f(jc=jc):
                    if not v_done[jc]:
                        v_chunk(jc)
                        v_done[jc] = True
                return [f]

            def KQ(h, i, wsel, ab=False):
                return [lambda: one_proj(h, i, wsel, act_bias=ab)]

            dues = {b: [] for b in range(16)}
            dues[0] = (KQ(0, 1, 2) + KQ(0, 2, 2) + KQ(0, 3, 2)
                       + KQ(1, 0, 2) + KQ(1, 1, 2) + Q_proj(1, 0)
                       + KQ(1, 2, 2) + KQ(1, 3, 2))
            dues[1] = (KQ(2, 0, 2) + V(0) + KQ(2, 1, 2) + Q_proj(2, 0)
                       + KQ(2, 2, 2) + V(1) + KQ(2, 3, 2) + V(2))
            dues[2] = (V(3) + Q_proj(3, 0) + V(4) + KQ(3, 0, 2)
                       + V(5) + KQ(3, 1, 2))
            dues[3] = (KQ(3, 2, 2) + V(6) + KQ(3, 3, 2) + V(7) + V(8)
                       + V(9) + Q_proj(0, 1))
            dues[4] = (V(10) + V(11) + Q_proj(1, 1) + V(12)
                       + Q_proj(2, 1) + V(13) + Q_proj(3, 1))
            dues[5] = V(14) + V(15) + Q_proj(0, 2) + Q_proj(1, 2)
            dues[6] = Q_proj(2, 2) + Q_proj(3, 2)
            dues[9] = Q_proj(0, 3) + Q_proj(1, 3)
            dues[10] = Q_proj(2, 3) + Q_proj(3, 3)

            def pv_rate(b):
                if b < 5:
                    return 0
                if b < 6:
                    return 1
                if b < 11:
                    return 2
                return 2 if (step_no[0] % 2 == 0) else 1

            # ---------------- prologue ----------------
            one_proj(0, 0, 2, act_bias=True)  # K(0,0) -- wkp arrives first
            one_proj(0, 0, 1, act_bias=True)  # Q(0,0)

            # ---------------- main loop ----------------
            for b in range(16):
                due = dues[b]
                # spread due items across the 8 steps
                for jp in range(8):
                    emit_S_exp(b, jp)
                    lo = (jp * len(due)) // 8
                    hi = ((jp + 1) * len(due)) // 8
                    for it in due[lo:hi]:
                        it()
                    drain_pv(pv_rate(b))
                    drain_norms()
                    if b >= 6:
                        drain_ofill(107 if b < 10 else 321)
                    step_no[0] += 1

            # ---------------- tail ----------------
            while pv_ready:
                b, jp, es = pv_ready.popleft()
                emit_PV(b, jp)
                if jp == 7:
                    norm_due[b] = True
            drain_norms()
            # remaining O items (quarter 2 leftovers + q3 pair01 leftovers)
            drain_ofill(10**9)
            # quarter 3, head pair (2,3) + combine with pair01 partial
            isl = slice(1536, 2048)
            y3 = ypool.tile([128, KC, 512], bf, tag="ysb", name="ysbq3")
            # six concurrent accumulators: ps1 x2, pss x2 (S banks are dead
            # after the last exp), psu x2 (UT banks dead after the last norm)
            pys = []
            for mc in range(KC):
                if mc < 2:
                    pys.append(ps1.tile([128, 512], f32, tag="p1",
                                        name=f"pyq3b{mc}"))
                elif mc < 4:
                    ts = pss.tile([128, 2, 512], f32, tag="s",
                                  name=f"pyq3b{mc}")
                    pys.append(ts[:, 0, :])
                else:
                    pys.append(psu.tile([128, 512], f32, tag="u",
                                        name=f"pyq3b{mc}"))
            terms = [(wo8_sb, out8_sb), (wor_sb, out8_sb), (wo8_sb, outr_sb)]
            for mc in range(KC):
                for i in (0, 1):
                    wsb, osb = terms[i]
                    nc.tensor.matmul(
                        pys[mc],
                        wsb[:, 2:4, 128 * mc : 128 * mc + 128],
                        osb[:, 2:4, isl],
                        start=(i == 0),
                        stop=False,
                        perf_mode=DR,
                    )
            for mc in range(KC):
                wsb, osb = terms[2]
                nc.tensor.matmul(
                    pys[mc],
                    wsb[:, 2:4, 128 * mc : 128 * mc + 128],
                    osb[:, 2:4, isl],
                    start=False,
                    stop=True,
                    perf_mode=DR,
                )
            for mc in range(KC):
                if mc % 2 == 0:
                    nc.scalar.activation(
                        y3[:, mc, :], pys[mc], mybir.ActivationFunctionType.Copy,
                        scale=YSCALE,
                    )
                else:
                    nc.vector.tensor_scalar_mul(y3[:, mc, :], pys[mc], YSCALE)
                    nc.sync.dma_start(
                        yT_d[:, mc - 1 : mc + 1, isl], y3[:, mc - 1 : mc + 1, :]
                    )

    _split_multi_waits(nc)
    return nc


def _split_multi_waits(nc):
    """Walrus codegen allows only ONE sync wait on most compute-instruction
    structs. Hoist extra waits onto standalone EventSemaphore instructions
    inserted just before the offender on the same engine (semantically
    identical for in-order engines). DMA descriptors (queue-dispatched) are
    left alone."""
    import bass_rust

    n_split = 0
    for f in nc.m.functions:
        for blk in f.blocks:
            il = blk.instructions
            i = 0
            while i < len(il):
                inst = il[i]
                try:
                    si = inst.sync_info
                    waits = list(si.on_wait)
                except Exception:
                    i += 1
                    continue
                if len(waits) > 1 and inst.engine != mybir.EngineType.Unassigned:
                    for w in waits[:-1]:
                        ev = mybir.InstEventSemaphore(
                            name=f"wsplit_{n_split}", ins=[], outs=[]
                        )
                        n_split += 1
                        ev.engine = inst.engine
                        ev.sync_info = bass_rust.SyncInfo(on_wait=[w], on_update=[])
                        il.insert(i, ev)
                        i += 1
                    inst.sync_info = bass_rust.SyncInfo(
                        on_wait=[waits[-1]], on_update=list(si.on_update)
                    )
                i += 1
    return n_split


def _get_nc():
    if "nc" not in _NC_CACHE:
        _NC_CACHE["nc"] = _build_bass()
    return _NC_CACHE["nc"]


def _q8pair(a):
    """f32 array -> (fp8, unscaled fp8 residual)"""
    a8 = a.astype(FP8)
    ar = (a - a8.astype(np.float32)).astype(FP8)
    return a8, ar


def _to_lhsT(w):
    """[384, 768] weight (rows = output dims) -> [128, KC, 384] f32 lhsT chunks."""
    return np.ascontiguousarray(w.T.reshape(KC, 128, 384).transpose(1, 0, 2))


def _prep_half(Wq, bq, Wk, bk, Wv, Wo, half):
    sl = slice(384 * half, 384 * (half + 1))
    wq8, wqr = _q8pair(_to_lhsT(Wq[sl, :].astype(np.float32) * SW))
    wk8, wkr = _q8pair(_to_lhsT(Wk[sl, :].astype(np.float32) * SW))
    wqp = np.concatenate([wq8, wqr], axis=2)
    wkp = np.concatenate([wk8, wkr], axis=2)

    wv8, wvr = _q8pair(_to_lhsT(Wv[sl, :].astype(np.float32) * SW))
    # pack [wv8 | wvr] along the chunk dim -> [128, 12, 384] -> [128, 6, 768]
    wvp = np.concatenate([wv8, wvr], axis=1).reshape(128, KC, 768)

    WoT = Wo[:, sl].T.astype(np.float32) * SW  # [384, 768]
    wo_pad = np.zeros((HC, 128, E), np.float32)
    for h in range(HC):
        wo_pad[h, 0:96] = WoT[96 * h : 96 * h + 96]
    wo8, wor = _q8pair(np.ascontiguousarray(wo_pad.transpose(1, 0, 2)))
    wop = np.concatenate([wo8, wor], axis=2)

    bqk = np.zeros((1, 2, HC, D), FP8)
    bb = np.asarray(bq)[sl].astype(np.float32) * SW
    b8 = bb.astype(FP8)
    br = (bb - b8.astype(np.float32)).astype(FP8)
    for h in range(HC):
        bqk[0, 0, h] = b8[96 * h : 96 * h + 96]
        bqk[0, 1, h] = br[96 * h : 96 * h + 96]

    return dict(wqp=wqp, wkp=wkp, wvp=wvp, wop=wop, bqk=bqk)


def _run(x, Wq, bq, Wk, bk, Wv, bv, Wo, bo, trace=False):
    x = np.asarray(x, dtype=np.float32)
    B = x.shape[0]
    halves = [
        _prep_half(np.asarray(Wq), np.asarray(bq), np.asarray(Wk),
                   np.asarray(bk), np.asarray(Wv), np.asarray(Wo), hf)
        for hf in range(2)
    ]
    xTs = []
    for b in range(B):
        xT = np.ascontiguousarray(x[b].T.reshape(KC, 128, NT).transpose(1, 0, 2))
        xTs.append(_q8pair(xT))

    in_maps = []
    for c in range(8):
        b, hf = c // 2, c % 2
        m = dict(halves[hf])
        m["x8"], m["xr"] = xTs[b]
        in_maps.append(m)

    nc = _get_nc()
    res = run_bass_kernel_spmd(nc, in_maps, core_ids=list(range(8)), trace=trace)

    # v-bias passes through softmax's convex weights: add bv @ Wo.T on host.
    bo32 = (np.asarray(bo, dtype=np.float32)
            + np.asarray(bv, dtype=np.float32)
            @ np.asarray(Wo, dtype=np.float32).T)
    y = np.empty((B, NT, E), np.float32)
    for b in range(B):
        p0 = res.results[2 * b]["yT"].astype(np.float32).transpose(1, 0, 2).reshape(E, NT)
        p1 = res.results[2 * b + 1]["yT"].astype(np.float32).transpose(1, 0, 2).reshape(E, NT)
        a0 = res.results[2 * b]["y3a"].astype(np.float32).transpose(1, 0, 2).reshape(E, 512)
        a1 = res.results[2 * b + 1]["y3a"].astype(np.float32).transpose(1, 0, 2).reshape(E, 512)
        y[b] = (p0 + p1).T + bo32
        y[b, 1536:2048, :] += (a0 + a1).T
    return y, res


def kernel(x, Wq, bq, Wk, bk, Wv, bv, Wo, bo):
    y, _ = _run(x, Wq, bq, Wk, bk, Wv, bv, Wo, bo, trace=False)
    return y
